# revision 29
# baseline (speedup 1.0000x reference)
"""Trainium2 Bass kernel for nn_BoostEnhancedAttention.

Reference computation:
    v   = (values @ W_v.T + b_v)                      # [B, NK, H*D_V]
    att = softmax(att3 ⊗ att12 interleaved, axis=k)   # [B, H, NQ, NK]
    out = (att @ v_per_head) @ W_o.T + b_o            # [B, NQ, D_MODEL]

Restructuring (exact algebra):
  - Scores factor as s[b,h,q,k] = att3[b,h,q,c(k)] * att12[b,h,...f(k)];
    E = exp(s) built by DVE mul + ACT exp on [c_loc, (f,h,q)] tiles.
  - W_v/W_o folded into per-head M_h = W_o[:,h] @ W_v[h,:], applied AFTER
    attention:  out[b] = sum_h (att_h @ values[b]) @ M_h.T + b_eff, so the
    AV matmul consumes `values` in natural [k, d] layout (k on partitions).
  - Softmax normalization deferred: G~ = E @ values accumulated
    unnormalized in PSUM; Z = column sums of E via ones-matmul (output
    replicated across partitions so the normalizing multiply needs no
    partition broadcast).

Schedule: per-ktile emission with the previous batch's epilogue (Z
matmul, reciprocal, unnormalized G stores) and the completed pair's
two-stage output projection (rank-64: A_h = G_h @ Wv_h^T via col-tiled
M=64 matmuls, then sum_h A_h @ Wo_h^T) interleaved INTO the next batch's
k-loop, so the PE FIFO never drains at batch boundaries.  Softmax
normalization is applied at the tiny v1 = A * (1/Z) step (per-column
scale), keeping Z/reciprocal off the critical path.  values arrive as 4
big DMAs per batch (partition = c_loc, free = (kt, d)), prefetched one
batch ahead.  esum uses two interleaved DVE chains folded into the Z
matmul accumulation group.

Sharding: data-parallel over batch, B=32 over 8 cores -> 4 batches/core.
No collectives; outputs concatenated on host.
"""

import numpy as np
import ml_dtypes

B, CH, CW, H, FH, FW = 32, 16, 16, 8, 4, 4
NQ = 64
NCELL = CH * CW          # 256 coarse cells (c)
F = FH * FW              # 16 fine positions per cell
NK = NCELL * F           # 4096
D_IN, D_V, D_MODEL = 512, 64, 512
N_CORES = 8
B_LOC = B // N_CORES     # 4
N_KT = 32                # k-tiles of 128: kt = half*16 + f, partition = c_loc
N_DT = 4                 # d_in tiles of 128
HQ = H * NQ              # 512
KT_CHUNK = 8             # k-tiles per values DMA chunk
N_CHUNK = N_KT // KT_CHUNK

BF16 = ml_dtypes.bfloat16


def _k_perm():
    """perm[k'] -> original k, where k' = (half*16+f)*128 + c_loc.

    Original key order is (ch, fh, cw, fw):  k = ch*256 + fh*64 + cw*4 + fw.
    New order groups a k-tile as (fixed f=(fh,fw), c = half*128 + c_loc).
    """
    perm = np.zeros(NK, np.int64)
    c = np.arange(NCELL)
    ch_i, cw_i = c // CW, c % CW
    for half in range(2):
        for f in range(F):
            kt = half * F + f
            fh, fw = f // FW, f % FW
            cc = half * 128 + np.arange(128)
            perm[kt * 128:(kt + 1) * 128] = (
                ch_i[cc] * (FH * CW * FW) + fh * (CW * FW) + cw_i[cc] * FW + fw
            )
    return perm


_PERM = _k_perm()
_NC_CACHE = {}


def _build_nc():
    from contextlib import ExitStack

    import concourse.bass as bass
    import concourse.tile as tile
    from concourse import bacc, mybir

    f32 = mybir.dt.float32
    bf16 = mybir.dt.bfloat16

    nc = bacc.Bacc("TRN2", target_bir_lowering=False, debug=False,
                   num_devices=N_CORES)

    # values_r[b, c_loc, kt*D_IN + d] = values[b, perm[kt*128+c_loc], d]
    values_r = nc.dram_tensor("values_r", [B_LOC, 128, N_KT * D_IN], bf16,
                              kind="ExternalInput")
    att3_t = nc.dram_tensor("att3_t", [B_LOC, NCELL, HQ], bf16,
                            kind="ExternalInput")
    att12_pair = nc.dram_tensor("att12_pair", [B_LOC, NCELL, F * H * 2], bf16,
                                kind="ExternalInput")
    wv_all = nc.dram_tensor("wv_all", [128, N_DT * H * D_V], bf16,
                            kind="ExternalInput")
    w2_all = nc.dram_tensor("w2_all", [128, (H // 2) * D_MODEL], bf16,
                            kind="ExternalInput")
    beff = nc.dram_tensor("beff", [1, D_MODEL], bf16, kind="ExternalInput")
    out = nc.dram_tensor("out", [B_LOC * NQ, D_MODEL], f32,
                         kind="ExternalOutput")

    with tile.TileContext(nc) as tc, ExitStack() as ctx:
        const_pool = ctx.enter_context(tc.tile_pool(name="const", bufs=1))
        a3_pool = ctx.enter_context(tc.tile_pool(name="a3", bufs=2))
        a12r_pool = ctx.enter_context(tc.tile_pool(name="a12r", bufs=2))
        vt_pool = ctx.enter_context(tc.tile_pool(name="vt", bufs=8))
        sc_pool = ctx.enter_context(tc.tile_pool(name="sc", bufs=3))
        et_pool = ctx.enter_context(tc.tile_pool(name="et", bufs=4))
        esum_pool = ctx.enter_context(tc.tile_pool(name="esum", bufs=2))
        zb_pool = ctx.enter_context(tc.tile_pool(name="zb", bufs=2))
        g_pool = ctx.enter_context(tc.tile_pool(name="gps", bufs=1, space="PSUM"))
        zo_pool = ctx.enter_context(tc.tile_pool(name="zops", bufs=1, space="PSUM"))
        o_sb_pool = ctx.enter_context(tc.tile_pool(name="osb", bufs=2))

        ones_sb = const_pool.tile([128, 128], bf16)
        nc.vector.memset(ones_sb[:], 1.0)
        warm_sb = const_pool.tile([128, D_MODEL], bf16, name="warm_sb")
        nc.gpsimd.memset(warm_sb[:], 1.0)
        beff_sb = const_pool.tile([1, D_MODEL], bf16)
        # g_all[d_loc, (dt, h, b, q)] : normalized attention output, bf16
        g_all = const_pool.tile([128, N_DT * H * B_LOC * NQ], bf16)
        wv_sb = const_pool.tile([128, N_DT * H * D_V], bf16, name="wv_sb")
        w2_sb = const_pool.tile([128, (H // 2) * D_MODEL], bf16, name="w2_sb")

        Q2 = NQ // 2

        # ---- per-batch state -----------------------------------------
        state = {}

        def emit_group(b, half, gi, FQ, f0, a3_t, a12r_t):
            """One score group: broadcast multiply + exp for FQ f-positions."""
            a3b = a3_t[half][:]
            in0 = bass.AP(a3b.tensor, a3b.offset,
                          [a3b.ap[0], [0, FQ], [NQ, H], [2, Q2], [1, 2]])
            sc = sc_pool.tile([128, 4 * HQ], bf16, tag="sc",
                              name=f"sc_{b}_{half}_{gi}")
            scb = sc[:]
            out_ap = bass.AP(scb.tensor, scb.offset,
                             [scb.ap[0], [HQ, FQ], [NQ, H], [2, Q2], [1, 2]])
            a12b = a12r_t[half][:]
            in1 = bass.AP(a12b.tensor, a12b.offset + f0 * H * 2,
                          [a12b.ap[0], [H * 2, FQ], [2, H], [0, Q2], [1, 2]])
            nc.vector.tensor_mul(out_ap, in0, in1)
            et = et_pool.tile([128, 4 * HQ], bf16, tag="et",
                              name=f"et_{b}_{half}_{gi}")
            nc.scalar.activation(et[:, :FQ * HQ], sc[:, :FQ * HQ],
                                 mybir.ActivationFunctionType.Exp)
            return et

        def dma_vt_chunk(b, j, pieces=1, ladder=None):
            """DMA chunk j (KT_CHUNK k-tiles) of batch b's values.

            ladder: list of piece widths in k-tiles (summing to KT_CHUNK);
            small leading pieces let the first matmuls start sooner."""
            vt = vt_pool.tile([128, KT_CHUNK * D_IN], bf16, tag="vt",
                              name=f"vt_{b}_{j}")
            src = values_r.ap()[b, :,
                                j * KT_CHUNK * D_IN:(j + 1) * KT_CHUNK * D_IN]
            if ladder is None:
                w = KT_CHUNK * D_IN // pieces
                widths = [w] * pieces
            else:
                widths = [x * D_IN for x in ladder]
            off = 0
            for w in widths:
                nc.sync.dma_start(vt[:, off:off + w], src[:, off:off + w])
                off += w
            state[("vt", b, j)] = vt

        def prologue(b):
            """a3/a12 DMAs + first score group(s) for batch b."""
            a3_t = [a3_pool.tile([128, HQ], bf16, tag=f"a3_{hf}",
                                 name=f"a3_{b}_{hf}") for hf in range(2)]
            for hf in range(2):
                nc.sync.dma_start(a3_t[hf][:],
                                  att3_t.ap()[b, hf * 128:(hf + 1) * 128, :])
            a12r_t = []
            for hf in range(2):
                a12r = a12r_pool.tile([128, F * H * 2], bf16, tag=f"a12r_{hf}",
                                      name=f"a12r_{b}_{hf}")
                nc.sync.dma_start(a12r[:],
                                  att12_pair.ap()[b, hf * 128:(hf + 1) * 128, :])
                a12r_t.append(a12r)
            groups = [1, 1, 2, 4, 4, 4] if b == 0 else [4, 4, 4, 4]
            et0 = emit_group(b, 0, 0, groups[0], 0, a3_t, a12r_t)
            state[("pro", b)] = (a3_t, a12r_t, groups, et0)

        def epilogue_part1(b):
            """Z matmul (from both partial esums) + reciprocal for batch b."""
            esA, esB = state[("esum", b)]
            zps = zo_pool.tile([128, HQ], f32, tag="zo", name=f"z_{b}")
            nc.tensor.matmul(zps[:], ones_sb[:], esA[:], start=True, stop=False)
            nc.tensor.matmul(zps[:], ones_sb[:], esB[:], start=False, stop=True)
            zb = zb_pool.tile([128, HQ], f32, name=f"zb_{b}")
            nc.vector.reciprocal_approx_fast(zb[:], zps[:])
            state[("zb", b)] = zb

        def gstore(b, dts, eng=0):
            """gps[dt] -> g_all columns for batch b (UNNORMALIZED).

            Normalization is deferred to the v1 step: stage 1 is per-head,
            so 1/Z is a per-column scale of the small A matrix.  eng=0 runs
            on ACT (keeps DVE free mid-stream); eng=1 on DVE (tail overlap)."""
            gps = state[("gps", b)]
            ga_v = g_all[:].rearrange("p (dt h bb q) -> p dt h bb q",
                                      dt=N_DT, h=H, bb=B_LOC)
            for dt in dts:
                if eng == 0:
                    nc.scalar.activation(
                        ga_v[:, dt, :, b, :],
                        gps[dt][:].rearrange("p (h q) -> p h q", h=H),
                        mybir.ActivationFunctionType.Copy)
                else:
                    nc.vector.tensor_copy(
                        ga_v[:, dt, :, b, :],
                        gps[dt][:].rearrange("p (h q) -> p h q", h=H))

        def zmat_build(bq):
            """zmat[p, (hp, b, q)] = 1/Z[2hp + parity(p), b, q] for the
            pair's two batches; rows 0-63 even h, 64-127 odd h."""
            zmat = zb_pool.tile([128, 4 * 128], f32, tag="zmat",
                                name=f"zmat_{bq}")
            for i in range(2):
                for par in range(2):
                    dstb = zmat[par * 64:par * 64 + 64, :]
                    dst = bass.AP(dstb.tensor, dstb.offset + i * NQ,
                                  [dstb.ap[0], [128, 4], [1, NQ]])
                    srcb = state[("zb", 2 * bq + i)][par * 64:par * 64 + 64, :]
                    src_ap = bass.AP(srcb.tensor, srcb.offset + par * NQ,
                                     [srcb.ap[0], [128, 4], [1, NQ]])
                    nc.vector.tensor_copy(dst, src_ap)
            state[("zmat", bq)] = zmat

        def stage1_mms(bq, dt):
            """A_h = sum_dt Wv_h[:, dt]^T @ Gn_h[dt] for all h; one dt block.

            Even h -> PE col group 0 (PSUM partitions 0-63), odd h -> col
            group 64 (partitions 64-127); h-pair hp gets columns hp*128."""
            aps = state.get(("aps", bq))
            if aps is None:
                aps = zo_pool.tile([128, 4 * 128], f32, tag="zo",
                                   name=f"aps_{bq}")
                state[("aps", bq)] = aps
            for h in range(H):
                col = dt * (H * B_LOC * NQ) + h * (B_LOC * NQ) + bq * 128
                po = (h % 2) * 64
                out_ap = aps[po:po + 64, (h // 2) * 128:(h // 2) * 128 + 128]
                # start=True clears has_written for the touched partition
                # rows across the FULL bank width, so it must fire exactly
                # once per row-group (h parity); later column regions rely
                # on per-element has_written==0 to overwrite on first hit.
                nc.tensor.matmul(
                    out_ap,
                    wv_sb[:, (dt * H + h) * D_V:(dt * H + h + 1) * D_V],
                    g_all[:, col:col + 128],
                    start=(dt == 0 and h < 2), stop=(dt == N_DT - 1),
                    tile_position=(0, po), skip_group_check=True)

        def stage1_copy(bq, part=None):
            v1 = state.get(("v1", bq))
            if v1 is None:
                v1 = o_sb_pool.tile([128, 4 * 128], bf16, tag="v1",
                                    name=f"v1_{bq}")
                state[("v1", bq)] = v1
            sl = slice(0, 512) if part is None else (
                slice(0, 256) if part == 0 else slice(256, 512))
            # normalize here: v1 = A * (1/Z) with per-column 1/Z
            nc.vector.tensor_mul(v1[:, sl], state[("aps", bq)][:, sl],
                                 state[("zmat", bq)][:, sl])

        def stage2_mms(bq, hps=range(4)):
            v1 = state[("v1", bq)]
            ops = state.get(("ops", bq))
            if ops is None:
                ops = zo_pool.tile([128, D_MODEL], f32, tag="zo",
                                   name=f"ops_{bq}")
                state[("ops", bq)] = ops
                # bias opens the accumulation group (depends on nothing),
                # so the group can close on the last stage-2 matmul
                nc.tensor.matmul(ops[:], ones_sb[0:1, :], beff_sb[:],
                                 start=True, stop=False)
            for hp in hps:
                nc.tensor.matmul(ops[:], v1[:, hp * 128:(hp + 1) * 128],
                                 w2_sb[:, hp * D_MODEL:(hp + 1) * D_MODEL],
                                 start=False, stop=(hp == H // 2 - 1))

        def mproj_finish(bq):
            ops = state[("ops", bq)]
            out_sb = o_sb_pool.tile([128, D_MODEL], f32, tag="osb",
                                    name=f"osb_{bq}")
            nc.scalar.activation(out_sb[:], ops[:],
                                 mybir.ActivationFunctionType.Copy)
            nc.sync.dma_start(out.ap()[bq * 128:(bq + 1) * 128, :], out_sb[:])

        # ---- startup: interleave values / a3 / a12 DMAs so kt0's data,
        # half-0 scores, and the E-chain all progress during the DMA ramp
        vt00 = vt_pool.tile([128, KT_CHUNK * D_IN], bf16, tag="vt",
                            name="vt_0_0")
        src00 = values_r.ap()[0, :, 0:KT_CHUNK * D_IN]
        state[("vt", 0, 0)] = vt00
        nc.sync.dma_start(vt00[:, :2 * D_IN], src00[:, :2 * D_IN])  # kt0-1
        a3_t0 = [a3_pool.tile([128, HQ], bf16, tag=f"a3_{hf}",
                              name=f"a3_0_{hf}") for hf in range(2)]
        a12r_t0 = [a12r_pool.tile([128, F * H * 2], bf16, tag=f"a12r_{hf}",
                                  name=f"a12r_0_{hf}") for hf in range(2)]
        nc.sync.dma_start(a3_t0[0][:], att3_t.ap()[0, 0:128, :])
        nc.sync.dma_start(a12r_t0[0][:], att12_pair.ap()[0, 0:128, :])
        nc.sync.dma_start(vt00[:, 2 * D_IN:4 * D_IN],
                          src00[:, 2 * D_IN:4 * D_IN])              # kt2-3
        nc.sync.dma_start(a3_t0[1][:], att3_t.ap()[0, 128:256, :])
        nc.sync.dma_start(a12r_t0[1][:], att12_pair.ap()[0, 128:256, :])
        nc.sync.dma_start(vt00[:, 4 * D_IN:8 * D_IN],
                          src00[:, 4 * D_IN:8 * D_IN])              # kt4-7
        groups0_b0 = [1, 1, 2, 4, 4, 4]
        et0_b0 = emit_group(0, 0, 0, groups0_b0[0], 0, a3_t0, a12r_t0)
        state[("pro", 0)] = (a3_t0, a12r_t0, groups0_b0, et0_b0)
        dma_vt_chunk(0, 1, pieces=2)
        dma_vt_chunk(0, 2, pieces=1)
        dma_vt_chunk(0, 3, pieces=1)

        warm = zo_pool.tile([128, D_MODEL], f32, tag="zo", name="warm")
        for wi in range(2):
            nc.tensor.matmul(warm[:, :128], ones_sb[:], ones_sb[:],
                             start=True, stop=True)
        for wi in range(9):
            nc.tensor.matmul(warm[:], ones_sb[:], warm_sb[:],
                             start=True, stop=True)

        DEFER = 8          # k-tiles whose dt3 matmul is deferred (b > 0)
        MPROJ_KT0 = 8      # first k-tile that carries output-proj matmuls

        for b in range(B_LOC):
            a3_t, a12r_t, groups0, et0 = state.pop(("pro", b))
            gps = [g_pool.tile([128, HQ], f32, tag=f"g{dt}", name=f"g_{b}_{dt}",
                               bufs=(2 if dt < 3 else 1))
                   for dt in range(N_DT)]
            state[("gps", b)] = gps
            esA = esum_pool.tile([128, HQ], bf16, tag="esA", name=f"esA_{b}")
            esB = esum_pool.tile([128, HQ], bf16, tag="esB", name=f"esB_{b}")
            state[("esum", b)] = (esA, esB)
            deferred = []
            # completed pair whose output projection runs during this batch
            bq = b // 2 - 1 if b % 2 == 0 else -1

            kt = 0
            for half in range(2):
                if half == 0:
                    groups = groups0
                elif b == B_LOC - 1:
                    # fine-grained trailing groups: the final exp/esum land
                    # sooner, shortening the tail epilogue chain
                    groups = [4, 4, 4, 2, 1, 1]
                else:
                    groups = [4, 4, 4, 4]
                f0 = 0
                for gi, FQ in enumerate(groups):
                    if half == 0 and gi == 0:
                        et = et0
                    else:
                        et = emit_group(b, half, gi, FQ, f0, a3_t, a12r_t)

                    for j in range(FQ):
                        # ---- hooks BEFORE this ktile's matmuls --------
                        if b > 0:
                            if kt == 4:
                                epilogue_part1(b - 1)
                                gstore(b - 1, [0, 1])
                            elif kt == 8:
                                gstore(b - 1, [2, 3])
                            elif kt == 10:
                                for dvt_ap, dets in deferred:
                                    nc.tensor.matmul(gps[3][:], dvt_ap, dets,
                                                     start=False, stop=False)
                                deferred = None  # catchup done
                        if b + 1 < B_LOC and kt % KT_CHUNK == 0:
                            dma_vt_chunk(b + 1, kt // KT_CHUNK)
                        if b == 0 and kt == 2:
                            nc.sync.dma_start(beff_sb[:], beff.ap())
                        if b == 1 and kt == 2:
                            nc.sync.dma_start(wv_sb[:], wv_all.ap())
                        if b == 1 and kt == 6:
                            nc.sync.dma_start(w2_sb[:], w2_all.ap())
                        if bq >= 0:
                            if kt == 6:
                                stage1_mms(bq, 0)
                            elif kt == 8:
                                pass  # normalize(b-1,[2,3]) emitted above
                            elif kt == 9:
                                stage1_mms(bq, 1)
                            elif kt == 11:
                                stage1_mms(bq, 2)
                            elif kt == 12:
                                stage1_mms(bq, 3)
                            elif kt == 10:
                                zmat_build(bq)
                            elif kt == 13:
                                stage1_copy(bq)
                            elif kt == 15:
                                stage2_mms(bq)
                                mproj_finish(bq)

                        # ---- AV matmuls for this ktile ----------------
                        vt = state[("vt", b, kt // KT_CHUNK)]
                        koff = (kt % KT_CHUNK) * D_IN
                        ets = et[:, j * HQ:(j + 1) * HQ]
                        start = kt == 0
                        stop = kt == N_KT - 1
                        for dt in range(3):
                            nc.tensor.matmul(
                                gps[dt][:],
                                vt[:, koff + dt * 128:koff + (dt + 1) * 128],
                                ets, start=start, stop=stop)
                        if b > 0 and kt < DEFER:
                            # dt3 bank still held by batch b-1's normalize
                            deferred.append(
                                (vt[:, koff + 3 * 128:koff + 4 * 128], ets))
                            if kt == 0:
                                # open the accumulation group on first MM
                                pass
                        else:
                            nc.tensor.matmul(
                                gps[3][:],
                                vt[:, koff + 3 * 128:koff + 4 * 128],
                                ets,
                                start=(kt == DEFER if b > 0 else start),
                                stop=stop)

                        # ---- esum: two interleaved chains on DVE ------
                        if kt == 0:
                            nc.vector.tensor_copy(esA[:], ets)
                        elif kt == 1:
                            nc.vector.tensor_copy(esB[:], ets)
                        elif kt % 2 == 0:
                            nc.vector.tensor_add(esA[:], esA[:], ets)
                        else:
                            nc.vector.tensor_add(esB[:], esB[:], ets)
                        kt += 1
                    f0 += FQ

            if b + 1 < B_LOC:
                prologue(b + 1)

        # ---- tail: epilogue of last batch + last pair's projection ----
        b_last = B_LOC - 1
        pq = B_LOC // 2 - 1
        epilogue_part1(b_last)
        zmat_build(pq)
        for dt in range(N_DT):
            gstore(b_last, [dt])
            stage1_mms(pq, dt)
        stage1_copy(pq, part=0)
        stage2_mms(pq, hps=range(0, 2))
        stage1_copy(pq, part=1)
        stage2_mms(pq, hps=range(2, 4))
        mproj_finish(pq)

    nc.compile()
    return nc


def _get_nc():
    if "nc" not in _NC_CACHE:
        _NC_CACHE["nc"] = _build_nc()
    return _NC_CACHE["nc"]


def _host_prep(att12, att3, values, W_v, b_v, W_o, b_o):
    att12 = np.asarray(att12, np.float32)
    att3 = np.asarray(att3, np.float32)
    values = np.asarray(values, np.float32)
    W_v = np.asarray(W_v, np.float32)
    b_v = np.asarray(b_v, np.float32)
    W_o = np.asarray(W_o, np.float32)
    b_o = np.asarray(b_o, np.float32)

    values_p = values[:, _PERM, :].astype(BF16)          # [B, 4096, 512]
    values_r = np.ascontiguousarray(
        values_p.reshape(B, N_KT, 128, D_IN).transpose(0, 2, 1, 3)
        .reshape(B, 128, N_KT * D_IN))
    att3_t = np.ascontiguousarray(
        att3.transpose(0, 3, 1, 2).reshape(B, NCELL, HQ)).astype(BF16)
    att12_r = np.ascontiguousarray(
        att12.transpose(0, 1, 2, 4, 5, 3).reshape(B, NCELL, F * H)).astype(BF16)
    att12_pair = np.ascontiguousarray(np.broadcast_to(
        att12_r[:, :, :, None], (B, NCELL, F * H, 2)).reshape(
        B, NCELL, F * H * 2))

    # Two-stage projection: A_h = Gn_h @ Wv_h^T, out = sum_h A_h @ Wo_h^T
    Wv3 = W_v.reshape(H, D_V, D_IN)
    Wo3 = W_o.reshape(D_MODEL, H, D_V)
    # wv_all[d_loc, (dt, h, dv)] = Wv3[h, dv, dt*128 + d_loc]
    wv_all = np.ascontiguousarray(
        Wv3.transpose(2, 0, 1).reshape(N_DT, 128, H, D_V)
        .transpose(1, 0, 2, 3).reshape(128, N_DT * H * D_V)).astype(BF16)
    # w2_all[p, (hp, dm)]: p<64 -> h=2hp, dv=p; p>=64 -> h=2hp+1, dv=p-64
    w2 = np.zeros((128, H // 2, D_MODEL), np.float32)
    for hp in range(H // 2):
        w2[0:64, hp, :] = Wo3[:, 2 * hp, :].T
        w2[64:128, hp, :] = Wo3[:, 2 * hp + 1, :].T
    w2_all = np.ascontiguousarray(
        w2.reshape(128, (H // 2) * D_MODEL)).astype(BF16)

    b_eff = b_o + np.einsum("dhv,hv->d", Wo3, b_v.reshape(H, D_V))
    beff = b_eff.reshape(1, D_MODEL).astype(BF16)
    return values_r, att3_t, att12_pair, wv_all, w2_all, beff


def kernel(att12, att3, values, W_v, b_v, W_o, b_o):
    from concourse.bass_utils import run_bass_kernel_spmd

    values_r, att3_t, att12_pair, wv_all, w2_all, beff = _host_prep(
        att12, att3, values, W_v, b_v, W_o, b_o)

    in_maps = []
    for core in range(N_CORES):
        s = slice(core * B_LOC, (core + 1) * B_LOC)
        in_maps.append({
            "values_r": np.ascontiguousarray(values_r[s]),
            "att3_t": np.ascontiguousarray(att3_t[s]),
            "att12_pair": np.ascontiguousarray(att12_pair[s]),
            "wv_all": wv_all,
            "w2_all": w2_all,
            "beff": beff,
        })

    nc = _get_nc()
    res = run_bass_kernel_spmd(nc, in_maps, core_ids=list(range(N_CORES)))
    out = np.concatenate(
        [res.results[i]["out"].reshape(B_LOC, NQ, D_MODEL)
         for i in range(N_CORES)], axis=0)
    return out.astype(np.float32)


# revision 31
# speedup vs baseline: 1.0042x; 1.0042x over previous
"""Trainium2 Bass kernel for nn_BoostEnhancedAttention.

Reference computation:
    v   = (values @ W_v.T + b_v)                      # [B, NK, H*D_V]
    att = softmax(att3 ⊗ att12 interleaved, axis=k)   # [B, H, NQ, NK]
    out = (att @ v_per_head) @ W_o.T + b_o            # [B, NQ, D_MODEL]

Restructuring (exact algebra):
  - Scores factor as s[b,h,q,k] = att3[b,h,q,c(k)] * att12[b,h,...f(k)];
    E = exp(s) built by DVE mul + ACT exp on [c_loc, (f,h,q)] tiles.
  - W_v/W_o folded into per-head M_h = W_o[:,h] @ W_v[h,:], applied AFTER
    attention:  out[b] = sum_h (att_h @ values[b]) @ M_h.T + b_eff, so the
    AV matmul consumes `values` in natural [k, d] layout (k on partitions).
  - Softmax normalization deferred: G~ = E @ values accumulated
    unnormalized in PSUM; Z = column sums of E via ones-matmul (output
    replicated across partitions so the normalizing multiply needs no
    partition broadcast).

Schedule: per-ktile emission with the previous batch's epilogue (Z
matmul, reciprocal, unnormalized G stores) and the completed pair's
two-stage output projection (rank-64: A_h = G_h @ Wv_h^T via col-tiled
M=64 matmuls, then sum_h A_h @ Wo_h^T) interleaved INTO the next batch's
k-loop, so the PE FIFO never drains at batch boundaries.  Softmax
normalization is applied at the tiny v1 = A * (1/Z) step (per-column
scale), keeping Z/reciprocal off the critical path.  values arrive as 4
big DMAs per batch (partition = c_loc, free = (kt, d)), prefetched one
batch ahead.  esum uses two interleaved DVE chains folded into the Z
matmul accumulation group.

Sharding: data-parallel over batch, B=32 over 8 cores -> 4 batches/core.
No collectives; outputs concatenated on host.
"""

import numpy as np
import ml_dtypes

B, CH, CW, H, FH, FW = 32, 16, 16, 8, 4, 4
NQ = 64
NCELL = CH * CW          # 256 coarse cells (c)
F = FH * FW              # 16 fine positions per cell
NK = NCELL * F           # 4096
D_IN, D_V, D_MODEL = 512, 64, 512
N_CORES = 8
B_LOC = B // N_CORES     # 4
N_KT = 32                # k-tiles of 128: kt = half*16 + f, partition = c_loc
N_DT = 4                 # d_in tiles of 128
HQ = H * NQ              # 512
KT_CHUNK = 8             # k-tiles per values DMA chunk
N_CHUNK = N_KT // KT_CHUNK

BF16 = ml_dtypes.bfloat16


def _k_perm():
    """perm[k'] -> original k, where k' = (half*16+f)*128 + c_loc.

    Original key order is (ch, fh, cw, fw):  k = ch*256 + fh*64 + cw*4 + fw.
    New order groups a k-tile as (fixed f=(fh,fw), c = half*128 + c_loc).
    """
    perm = np.zeros(NK, np.int64)
    c = np.arange(NCELL)
    ch_i, cw_i = c // CW, c % CW
    for half in range(2):
        for f in range(F):
            kt = half * F + f
            fh, fw = f // FW, f % FW
            cc = half * 128 + np.arange(128)
            perm[kt * 128:(kt + 1) * 128] = (
                ch_i[cc] * (FH * CW * FW) + fh * (CW * FW) + cw_i[cc] * FW + fw
            )
    return perm


_PERM = _k_perm()
_NC_CACHE = {}


def _build_nc():
    from contextlib import ExitStack

    import concourse.bass as bass
    import concourse.tile as tile
    from concourse import bacc, mybir

    f32 = mybir.dt.float32
    bf16 = mybir.dt.bfloat16

    nc = bacc.Bacc("TRN2", target_bir_lowering=False, debug=False,
                   num_devices=N_CORES)

    # values_r[b, c_loc, kt*D_IN + d] = values[b, perm[kt*128+c_loc], d]
    values_r = nc.dram_tensor("values_r", [B_LOC, 128, N_KT * D_IN], bf16,
                              kind="ExternalInput")
    att3_t = nc.dram_tensor("att3_t", [B_LOC, NCELL, HQ], bf16,
                            kind="ExternalInput")
    att12_pair = nc.dram_tensor("att12_pair", [B_LOC, NCELL, F * H * 2], bf16,
                                kind="ExternalInput")
    wv_all = nc.dram_tensor("wv_all", [128, N_DT * H * D_V], bf16,
                            kind="ExternalInput")
    w2_all = nc.dram_tensor("w2_all", [128, (H // 2) * D_MODEL], bf16,
                            kind="ExternalInput")
    beff = nc.dram_tensor("beff", [1, D_MODEL], bf16, kind="ExternalInput")
    out = nc.dram_tensor("out", [B_LOC * NQ, D_MODEL], f32,
                         kind="ExternalOutput")

    with tile.TileContext(nc) as tc, ExitStack() as ctx:
        const_pool = ctx.enter_context(tc.tile_pool(name="const", bufs=1))
        a3_pool = ctx.enter_context(tc.tile_pool(name="a3", bufs=2))
        a12r_pool = ctx.enter_context(tc.tile_pool(name="a12r", bufs=2))
        vt_pool = ctx.enter_context(tc.tile_pool(name="vt", bufs=8))
        sc_pool = ctx.enter_context(tc.tile_pool(name="sc", bufs=3))
        et_pool = ctx.enter_context(tc.tile_pool(name="et", bufs=4))
        esum_pool = ctx.enter_context(tc.tile_pool(name="esum", bufs=2))
        zb_pool = ctx.enter_context(tc.tile_pool(name="zb", bufs=2))
        g_pool = ctx.enter_context(tc.tile_pool(name="gps", bufs=1, space="PSUM"))
        zo_pool = ctx.enter_context(tc.tile_pool(name="zops", bufs=1, space="PSUM"))
        o_sb_pool = ctx.enter_context(tc.tile_pool(name="osb", bufs=2))

        ones_sb = const_pool.tile([128, 128], bf16)
        nc.vector.memset(ones_sb[:], 1.0)
        warm_sb = const_pool.tile([128, D_MODEL], bf16, name="warm_sb")
        nc.gpsimd.memset(warm_sb[:], 1.0)
        beff_sb = const_pool.tile([1, D_MODEL], bf16)
        # g_all[d_loc, (dt, h, b, q)] : normalized attention output, bf16
        g_all = const_pool.tile([128, N_DT * H * B_LOC * NQ], bf16)
        wv_sb = const_pool.tile([128, N_DT * H * D_V], bf16, name="wv_sb")
        w2_sb = const_pool.tile([128, (H // 2) * D_MODEL], bf16, name="w2_sb")

        Q2 = NQ // 2

        # ---- per-batch state -----------------------------------------
        state = {}

        def emit_group(b, half, gi, FQ, f0, a3_t, a12r_t):
            """One score group: broadcast multiply + exp for FQ f-positions."""
            a3b = a3_t[half][:]
            in0 = bass.AP(a3b.tensor, a3b.offset,
                          [a3b.ap[0], [0, FQ], [NQ, H], [2, Q2], [1, 2]])
            sc = sc_pool.tile([128, 4 * HQ], bf16, tag="sc",
                              name=f"sc_{b}_{half}_{gi}")
            scb = sc[:]
            out_ap = bass.AP(scb.tensor, scb.offset,
                             [scb.ap[0], [HQ, FQ], [NQ, H], [2, Q2], [1, 2]])
            a12b = a12r_t[half][:]
            in1 = bass.AP(a12b.tensor, a12b.offset + f0 * H * 2,
                          [a12b.ap[0], [H * 2, FQ], [2, H], [0, Q2], [1, 2]])
            nc.vector.tensor_mul(out_ap, in0, in1)
            et = et_pool.tile([128, 4 * HQ], bf16, tag="et",
                              name=f"et_{b}_{half}_{gi}")
            nc.scalar.activation(et[:, :FQ * HQ], sc[:, :FQ * HQ],
                                 mybir.ActivationFunctionType.Exp)
            return et

        def dma_vt_chunk(b, j, pieces=1, ladder=None):
            """DMA chunk j (KT_CHUNK k-tiles) of batch b's values.

            ladder: list of piece widths in k-tiles (summing to KT_CHUNK);
            small leading pieces let the first matmuls start sooner."""
            vt = vt_pool.tile([128, KT_CHUNK * D_IN], bf16, tag="vt",
                              name=f"vt_{b}_{j}")
            src = values_r.ap()[b, :,
                                j * KT_CHUNK * D_IN:(j + 1) * KT_CHUNK * D_IN]
            if ladder is None:
                w = KT_CHUNK * D_IN // pieces
                widths = [w] * pieces
            else:
                widths = [x * D_IN for x in ladder]
            off = 0
            for w in widths:
                nc.sync.dma_start(vt[:, off:off + w], src[:, off:off + w])
                off += w
            state[("vt", b, j)] = vt

        def prologue(b):
            """a3/a12 DMAs + first score group(s) for batch b."""
            a3_t = [a3_pool.tile([128, HQ], bf16, tag=f"a3_{hf}",
                                 name=f"a3_{b}_{hf}") for hf in range(2)]
            for hf in range(2):
                nc.sync.dma_start(a3_t[hf][:],
                                  att3_t.ap()[b, hf * 128:(hf + 1) * 128, :])
            a12r_t = []
            for hf in range(2):
                a12r = a12r_pool.tile([128, F * H * 2], bf16, tag=f"a12r_{hf}",
                                      name=f"a12r_{b}_{hf}")
                nc.sync.dma_start(a12r[:],
                                  att12_pair.ap()[b, hf * 128:(hf + 1) * 128, :])
                a12r_t.append(a12r)
            groups = [1, 1, 2, 4, 4, 4] if b == 0 else [4, 4, 4, 4]
            et0 = emit_group(b, 0, 0, groups[0], 0, a3_t, a12r_t)
            state[("pro", b)] = (a3_t, a12r_t, groups, et0)

        def epilogue_part1(b):
            """Z matmul + reciprocal written straight into the pair's zmat
            layout: zmat[p, (hp, b%2, q)] = 1/Z[2hp + parity(p), q]."""
            esA, esB = state[("esum", b)]
            zps = zo_pool.tile([128, HQ], f32, tag="zo", name=f"z_{b}")
            nc.tensor.matmul(zps[:], ones_sb[:], esA[:], start=True, stop=False)
            nc.tensor.matmul(zps[:], ones_sb[:], esB[:], start=False, stop=True)
            bq, i = b // 2, b % 2
            zb = zb_pool.tile([128, HQ], f32, tag="zb", name=f"zb_{b}")
            nc.vector.reciprocal_approx_fast(zb[:], zps[:])
            zmat = state.get(("zmat", bq))
            if zmat is None:
                zmat = zb_pool.tile([128, 4 * 128], f32, tag="zmat",
                                    name=f"zmat_{bq}")
                state[("zmat", bq)] = zmat
            for par in range(2):
                dstb = zmat[par * 64:par * 64 + 64, :]
                dst = bass.AP(dstb.tensor, dstb.offset + i * NQ,
                              [dstb.ap[0], [128, 4], [1, NQ]])
                srcb = zb[par * 64:par * 64 + 64, :]
                src_ap = bass.AP(srcb.tensor, srcb.offset + par * NQ,
                                 [srcb.ap[0], [128, 4], [1, NQ]])
                nc.vector.tensor_copy(dst, src_ap)

        def gstore(b, dts, eng=0):
            """gps[dt] -> g_all columns for batch b (UNNORMALIZED).

            Normalization is deferred to the v1 step: stage 1 is per-head,
            so 1/Z is a per-column scale of the small A matrix.  eng=0 runs
            on ACT (keeps DVE free mid-stream); eng=1 on DVE (tail overlap)."""
            gps = state[("gps", b)]
            ga_v = g_all[:].rearrange("p (dt h bb q) -> p dt h bb q",
                                      dt=N_DT, h=H, bb=B_LOC)
            for dt in dts:
                if eng == 0:
                    nc.scalar.activation(
                        ga_v[:, dt, :, b, :],
                        gps[dt][:].rearrange("p (h q) -> p h q", h=H),
                        mybir.ActivationFunctionType.Copy)
                else:
                    nc.vector.tensor_copy(
                        ga_v[:, dt, :, b, :],
                        gps[dt][:].rearrange("p (h q) -> p h q", h=H))


        def stage1_mms(bq, dt):
            """A_h = sum_dt Wv_h[:, dt]^T @ Gn_h[dt] for all h; one dt block.

            Even h -> PE col group 0 (PSUM partitions 0-63), odd h -> col
            group 64 (partitions 64-127); h-pair hp gets columns hp*128."""
            aps = state.get(("aps", bq))
            if aps is None:
                aps = zo_pool.tile([128, 4 * 128], f32, tag="zo",
                                   name=f"aps_{bq}")
                state[("aps", bq)] = aps
            for h in range(H):
                col = dt * (H * B_LOC * NQ) + h * (B_LOC * NQ) + bq * 128
                po = (h % 2) * 64
                out_ap = aps[po:po + 64, (h // 2) * 128:(h // 2) * 128 + 128]
                # start=True clears has_written for the touched partition
                # rows across the FULL bank width, so it must fire exactly
                # once per row-group (h parity); later column regions rely
                # on per-element has_written==0 to overwrite on first hit.
                nc.tensor.matmul(
                    out_ap,
                    wv_sb[:, (dt * H + h) * D_V:(dt * H + h + 1) * D_V],
                    g_all[:, col:col + 128],
                    start=(dt == 0 and h < 2), stop=(dt == N_DT - 1),
                    tile_position=(0, po), skip_group_check=True)

        def stage1_copy(bq, part=None):
            v1 = state.get(("v1", bq))
            if v1 is None:
                v1 = o_sb_pool.tile([128, 4 * 128], bf16, tag="v1",
                                    name=f"v1_{bq}")
                state[("v1", bq)] = v1
            sl = slice(0, 512) if part is None else (
                slice(0, 256) if part == 0 else slice(256, 512))
            # normalize here: v1 = A * (1/Z) with per-column 1/Z
            nc.vector.tensor_mul(v1[:, sl], state[("aps", bq)][:, sl],
                                 state[("zmat", bq)][:, sl])

        def stage2_mms(bq, hps=range(4)):
            v1 = state[("v1", bq)]
            ops = state.get(("ops", bq))
            if ops is None:
                ops = zo_pool.tile([128, D_MODEL], f32, tag="zo",
                                   name=f"ops_{bq}")
                state[("ops", bq)] = ops
                # bias opens the accumulation group (depends on nothing),
                # so the group can close on the last stage-2 matmul
                nc.tensor.matmul(ops[:], ones_sb[0:1, :], beff_sb[:],
                                 start=True, stop=False)
            for hp in hps:
                nc.tensor.matmul(ops[:], v1[:, hp * 128:(hp + 1) * 128],
                                 w2_sb[:, hp * D_MODEL:(hp + 1) * D_MODEL],
                                 start=False, stop=(hp == H // 2 - 1))

        def mproj_finish(bq):
            ops = state[("ops", bq)]
            out_sb = o_sb_pool.tile([128, D_MODEL], f32, tag="osb",
                                    name=f"osb_{bq}")
            nc.scalar.activation(out_sb[:], ops[:],
                                 mybir.ActivationFunctionType.Copy)
            nc.sync.dma_start(out.ap()[bq * 128:(bq + 1) * 128, :], out_sb[:])

        # ---- startup: interleave values / a3 / a12 DMAs so kt0's data,
        # half-0 scores, and the E-chain all progress during the DMA ramp
        vt00 = vt_pool.tile([128, KT_CHUNK * D_IN], bf16, tag="vt",
                            name="vt_0_0")
        src00 = values_r.ap()[0, :, 0:KT_CHUNK * D_IN]
        state[("vt", 0, 0)] = vt00
        nc.sync.dma_start(vt00[:, :2 * D_IN], src00[:, :2 * D_IN])  # kt0-1
        a3_t0 = [a3_pool.tile([128, HQ], bf16, tag=f"a3_{hf}",
                              name=f"a3_0_{hf}") for hf in range(2)]
        a12r_t0 = [a12r_pool.tile([128, F * H * 2], bf16, tag=f"a12r_{hf}",
                                  name=f"a12r_0_{hf}") for hf in range(2)]
        nc.sync.dma_start(a3_t0[0][:], att3_t.ap()[0, 0:128, :])
        nc.sync.dma_start(a12r_t0[0][:], att12_pair.ap()[0, 0:128, :])
        nc.sync.dma_start(vt00[:, 2 * D_IN:4 * D_IN],
                          src00[:, 2 * D_IN:4 * D_IN])              # kt2-3
        nc.sync.dma_start(a3_t0[1][:], att3_t.ap()[0, 128:256, :])
        nc.sync.dma_start(a12r_t0[1][:], att12_pair.ap()[0, 128:256, :])
        nc.sync.dma_start(vt00[:, 4 * D_IN:8 * D_IN],
                          src00[:, 4 * D_IN:8 * D_IN])              # kt4-7
        groups0_b0 = [1, 1, 2, 4, 4, 4]
        et0_b0 = emit_group(0, 0, 0, groups0_b0[0], 0, a3_t0, a12r_t0)
        state[("pro", 0)] = (a3_t0, a12r_t0, groups0_b0, et0_b0)
        dma_vt_chunk(0, 1, pieces=2)
        dma_vt_chunk(0, 2, pieces=1)
        dma_vt_chunk(0, 3, pieces=1)

        warm = zo_pool.tile([128, D_MODEL], f32, tag="zo", name="warm")
        for wi in range(2):
            nc.tensor.matmul(warm[:, :128], ones_sb[:], ones_sb[:],
                             start=True, stop=True)
        for wi in range(9):
            nc.tensor.matmul(warm[:], ones_sb[:], warm_sb[:],
                             start=True, stop=True)

        DEFER = 8          # k-tiles whose dt3 matmul is deferred (b > 0)
        MPROJ_KT0 = 8      # first k-tile that carries output-proj matmuls

        for b in range(B_LOC):
            a3_t, a12r_t, groups0, et0 = state.pop(("pro", b))
            gps = [g_pool.tile([128, HQ], f32, tag=f"g{dt}", name=f"g_{b}_{dt}",
                               bufs=(2 if dt < 3 else 1))
                   for dt in range(N_DT)]
            state[("gps", b)] = gps
            esA = esum_pool.tile([128, HQ], bf16, tag="esA", name=f"esA_{b}")
            esB = esum_pool.tile([128, HQ], bf16, tag="esB", name=f"esB_{b}")
            state[("esum", b)] = (esA, esB)
            deferred = []
            # completed pair whose output projection runs during this batch
            bq = b // 2 - 1 if b % 2 == 0 else -1

            kt = 0
            for half in range(2):
                if half == 0:
                    groups = groups0
                elif b == B_LOC - 1:
                    # fine-grained trailing groups: the final exp/esum land
                    # sooner, shortening the tail epilogue chain
                    groups = [4, 4, 4, 2, 1, 1]
                else:
                    groups = [4, 4, 4, 4]
                f0 = 0
                for gi, FQ in enumerate(groups):
                    if half == 0 and gi == 0:
                        et = et0
                    else:
                        et = emit_group(b, half, gi, FQ, f0, a3_t, a12r_t)

                    for j in range(FQ):
                        # ---- hooks BEFORE this ktile's matmuls --------
                        if b > 0:
                            if kt == 4:
                                epilogue_part1(b - 1)
                                gstore(b - 1, [0, 1])
                            elif kt == 8:
                                gstore(b - 1, [2, 3])
                            elif kt == 10:
                                for dvt_ap, dets in deferred:
                                    nc.tensor.matmul(gps[3][:], dvt_ap, dets,
                                                     start=False, stop=False)
                                deferred = None  # catchup done
                        if b + 1 < B_LOC and kt % KT_CHUNK == 0:
                            dma_vt_chunk(b + 1, kt // KT_CHUNK)
                        if b == 0 and kt == 2:
                            nc.sync.dma_start(beff_sb[:], beff.ap())
                        if b == 1 and kt == 2:
                            nc.sync.dma_start(wv_sb[:], wv_all.ap())
                        if b == 1 and kt == 6:
                            nc.sync.dma_start(w2_sb[:], w2_all.ap())
                        if bq >= 0:
                            if kt == 6:
                                stage1_mms(bq, 0)
                            elif kt == 8:
                                pass  # normalize(b-1,[2,3]) emitted above
                            elif kt == 9:
                                stage1_mms(bq, 1)
                            elif kt == 11:
                                stage1_mms(bq, 2)
                            elif kt == 12:
                                stage1_mms(bq, 3)
                            elif kt == 13:
                                stage1_copy(bq)
                            elif kt == 15:
                                stage2_mms(bq)
                                mproj_finish(bq)

                        # ---- AV matmuls for this ktile ----------------
                        vt = state[("vt", b, kt // KT_CHUNK)]
                        koff = (kt % KT_CHUNK) * D_IN
                        ets = et[:, j * HQ:(j + 1) * HQ]
                        start = kt == 0
                        stop = kt == N_KT - 1
                        for dt in range(3):
                            nc.tensor.matmul(
                                gps[dt][:],
                                vt[:, koff + dt * 128:koff + (dt + 1) * 128],
                                ets, start=start, stop=stop)
                        if b > 0 and kt < DEFER:
                            # dt3 bank still held by batch b-1's normalize
                            deferred.append(
                                (vt[:, koff + 3 * 128:koff + 4 * 128], ets))
                            if kt == 0:
                                # open the accumulation group on first MM
                                pass
                        else:
                            nc.tensor.matmul(
                                gps[3][:],
                                vt[:, koff + 3 * 128:koff + 4 * 128],
                                ets,
                                start=(kt == DEFER if b > 0 else start),
                                stop=stop)

                        # ---- esum: two interleaved chains on DVE ------
                        if kt == 0:
                            nc.vector.tensor_copy(esA[:], ets)
                        elif kt == 1:
                            nc.vector.tensor_copy(esB[:], ets)
                        elif kt % 2 == 0:
                            nc.vector.tensor_add(esA[:], esA[:], ets)
                        else:
                            nc.vector.tensor_add(esB[:], esB[:], ets)
                        kt += 1
                    f0 += FQ

            if b + 1 < B_LOC:
                prologue(b + 1)

        # ---- tail: epilogue of last batch + last pair's projection ----
        b_last = B_LOC - 1
        pq = B_LOC // 2 - 1
        state[("aps", pq)] = g_pool.tile([128, 4 * 128], f32, tag="g0",
                                         name="aps_tail", bufs=2)
        epilogue_part1(b_last)
        for dt in range(N_DT):
            gstore(b_last, [dt])
            stage1_mms(pq, dt)
        stage1_copy(pq, part=0)
        stage2_mms(pq, hps=range(0, 2))
        stage1_copy(pq, part=1)
        stage2_mms(pq, hps=range(2, 4))
        mproj_finish(pq)

    nc.compile()
    return nc


def _get_nc():
    if "nc" not in _NC_CACHE:
        _NC_CACHE["nc"] = _build_nc()
    return _NC_CACHE["nc"]


def _host_prep(att12, att3, values, W_v, b_v, W_o, b_o):
    att12 = np.asarray(att12, np.float32)
    att3 = np.asarray(att3, np.float32)
    values = np.asarray(values, np.float32)
    W_v = np.asarray(W_v, np.float32)
    b_v = np.asarray(b_v, np.float32)
    W_o = np.asarray(W_o, np.float32)
    b_o = np.asarray(b_o, np.float32)

    values_p = values[:, _PERM, :].astype(BF16)          # [B, 4096, 512]
    values_r = np.ascontiguousarray(
        values_p.reshape(B, N_KT, 128, D_IN).transpose(0, 2, 1, 3)
        .reshape(B, 128, N_KT * D_IN))
    att3_t = np.ascontiguousarray(
        att3.transpose(0, 3, 1, 2).reshape(B, NCELL, HQ)).astype(BF16)
    att12_r = np.ascontiguousarray(
        att12.transpose(0, 1, 2, 4, 5, 3).reshape(B, NCELL, F * H)).astype(BF16)
    att12_pair = np.ascontiguousarray(np.broadcast_to(
        att12_r[:, :, :, None], (B, NCELL, F * H, 2)).reshape(
        B, NCELL, F * H * 2))

    # Two-stage projection: A_h = Gn_h @ Wv_h^T, out = sum_h A_h @ Wo_h^T
    Wv3 = W_v.reshape(H, D_V, D_IN)
    Wo3 = W_o.reshape(D_MODEL, H, D_V)
    # wv_all[d_loc, (dt, h, dv)] = Wv3[h, dv, dt*128 + d_loc]
    wv_all = np.ascontiguousarray(
        Wv3.transpose(2, 0, 1).reshape(N_DT, 128, H, D_V)
        .transpose(1, 0, 2, 3).reshape(128, N_DT * H * D_V)).astype(BF16)
    # w2_all[p, (hp, dm)]: p<64 -> h=2hp, dv=p; p>=64 -> h=2hp+1, dv=p-64
    w2 = np.zeros((128, H // 2, D_MODEL), np.float32)
    for hp in range(H // 2):
        w2[0:64, hp, :] = Wo3[:, 2 * hp, :].T
        w2[64:128, hp, :] = Wo3[:, 2 * hp + 1, :].T
    w2_all = np.ascontiguousarray(
        w2.reshape(128, (H // 2) * D_MODEL)).astype(BF16)

    b_eff = b_o + np.einsum("dhv,hv->d", Wo3, b_v.reshape(H, D_V))
    beff = b_eff.reshape(1, D_MODEL).astype(BF16)
    return values_r, att3_t, att12_pair, wv_all, w2_all, beff


def kernel(att12, att3, values, W_v, b_v, W_o, b_o):
    from concourse.bass_utils import run_bass_kernel_spmd

    values_r, att3_t, att12_pair, wv_all, w2_all, beff = _host_prep(
        att12, att3, values, W_v, b_v, W_o, b_o)

    in_maps = []
    for core in range(N_CORES):
        s = slice(core * B_LOC, (core + 1) * B_LOC)
        in_maps.append({
            "values_r": np.ascontiguousarray(values_r[s]),
            "att3_t": np.ascontiguousarray(att3_t[s]),
            "att12_pair": np.ascontiguousarray(att12_pair[s]),
            "wv_all": wv_all,
            "w2_all": w2_all,
            "beff": beff,
        })

    nc = _get_nc()
    res = run_bass_kernel_spmd(nc, in_maps, core_ids=list(range(N_CORES)))
    out = np.concatenate(
        [res.results[i]["out"].reshape(B_LOC, NQ, D_MODEL)
         for i in range(N_CORES)], axis=0)
    return out.astype(np.float32)
